# revision 1
# baseline (speedup 1.0000x reference)
"""JKNet (5-layer GCN + JumpingKnowledge-max + linear head) on 8 Trainium2 cores.

Strategy (dst-sharded message passing):
  - Nodes are sharded contiguously across 8 cores (12500 per core).
  - Edges (plus explicit self-loops carrying the 1/deg self term) are
    partitioned by destination shard, grouped by 128-row destination tile,
    and sorted by source inside each tile for gather locality.
  - Every core keeps a full replica of the current layer's node features in
    HBM (x for layer 0, AllGather output for later layers).  Aggregation for
    a destination tile: indirect-DMA gather of the source rows (batched over
    groups of tiles, ~2.5 MB per call), then per 128-edge chunk build a
    one-hot selection matrix S[e, dst] = norm_e * (iota == dst_e) with one
    fused DVE op, and accumulate q^T[feat, dst] += msgs^T @ S on the PE.
  - Per tile: q^T -> SBUF, W matmul (feature-major), fused BN+ReLU on ACT,
    JumpingKnowledge running max in SBUF, PE transpose back to node-major,
    DMA to the AllGather input buffer.
  - One 8-core AllGather per layer (except the last) rebuilds the replica.
  - Head: logits = hmax^T.T @ lin_w per tile, + bias, log_softmax along the
    free dim, DMA the core's [12500, 40] shard out.

The per-destination-tile chunk counts are data dependent; the Bass program
is generated per problem instance (shared by all 8 cores -- per-tile chunk
counts are maxed over cores and shorter cores are padded with zero-norm
edges).
"""

import math
import os

import numpy as np

import concourse.bass as bass
import concourse.mybir as mybir
import concourse.tile as tile
from concourse import bacc
from concourse.bass_utils import run_bass_kernel_spmd
from concourse.masks import make_identity

P = 128          # partitions / feature dim / edge-chunk size
NCORES = 8
BN_EPS = 1e-5
G_TILES = 4      # dst tiles per batched indirect gather (~2.5 MB fp32)


# ---------------------------------------------------------------- host prep
def preprocess_edges(edge_index, n_nodes, ncores=NCORES):
    """Partition edges (incl. self-loops) by destination shard.

    Returns (per_core, k_tiles):
      per_core: list of dicts with 'eidx' [P, K] int32, 'edst' [P, K] f32,
                'enrm' [P, K] f32  (K = sum of shared per-tile chunk counts)
      k_tiles:  list of per-dst-tile chunk counts (shared across cores)
    """
    row = np.asarray(edge_index[0], dtype=np.int64)   # dst
    col = np.asarray(edge_index[1], dtype=np.int64)   # src
    deg = np.bincount(row, minlength=n_nodes).astype(np.float64) + 1.0
    dinv = (1.0 / np.sqrt(deg)).astype(np.float32)

    dst_all = np.concatenate([row, np.arange(n_nodes, dtype=np.int64)])
    src_all = np.concatenate([col, np.arange(n_nodes, dtype=np.int64)])
    nrm_all = np.concatenate([dinv[row] * dinv[col], dinv * dinv]).astype(np.float32)

    sh = n_nodes // ncores
    t_tiles = math.ceil(sh / P)
    core_of = dst_all // sh

    cores = []
    counts = np.zeros((ncores, t_tiles), dtype=np.int64)
    for c in range(ncores):
        m = core_of == c
        d = (dst_all[m] - c * sh).astype(np.int64)
        s = src_all[m]
        w = nrm_all[m]
        tid = d // P
        din = (d % P).astype(np.float32)
        order = np.lexsort((s, tid))
        tid, din, s, w = tid[order], din[order], s[order], w[order]
        counts[c] = np.bincount(tid, minlength=t_tiles)
        cores.append((tid, din, s, w))

    k_tiles = [max(1, int(math.ceil(counts[:, t].max() / P))) for t in range(t_tiles)]
    k_total = sum(k_tiles)
    offs = np.concatenate([[0], np.cumsum(k_tiles)])

    per_core = []
    for c in range(ncores):
        tid, din, s, w = cores[c]
        idx_f = np.zeros(k_total * P, dtype=np.int32)
        dst_f = np.zeros(k_total * P, dtype=np.float32)
        nrm_f = np.zeros(k_total * P, dtype=np.float32)
        tstart = np.concatenate([[0], np.cumsum(counts[c])])
        for t in range(t_tiles):
            n = int(counts[c][t])
            a, b = int(tstart[t]), int(tstart[t] + n)
            o = int(offs[t]) * P
            idx_f[o:o + n] = s[a:b]
            dst_f[o:o + n] = din[a:b]
            nrm_f[o:o + n] = w[a:b]
            pad = k_tiles[t] * P - n
            if pad and n:
                idx_f[o + n:o + n + pad] = s[b - 1]   # repeat last src: locality
        per_core.append({
            "eidx": np.ascontiguousarray(idx_f.reshape(k_total, P).T),
            "edst": np.ascontiguousarray(dst_f.reshape(k_total, P).T),
            "enrm": np.ascontiguousarray(nrm_f.reshape(k_total, P).T),
        })
    return per_core, k_tiles


# ---------------------------------------------------------------- program
def build_program(n_nodes, n_layers, n_cls, k_tiles, ncores=NCORES):
    f32 = mybir.dt.float32
    sh = n_nodes // ncores
    t_tiles = math.ceil(sh / P)
    k_total = sum(k_tiles)
    offs = np.concatenate([[0], np.cumsum(k_tiles)])
    groups = [list(range(g * G_TILES, min((g + 1) * G_TILES, t_tiles)))
              for g in range(math.ceil(t_tiles / G_TILES))]
    max_cg = max(int(offs[g[-1] + 1] - offs[g[0]]) for g in groups)

    nc = bacc.Bacc("TRN2", target_bir_lowering=False, debug=False,
                   num_devices=ncores)
    x_t = nc.dram_tensor("x", [n_nodes, P], f32, kind="ExternalInput")
    idx_t = nc.dram_tensor("eidx", [P, k_total], mybir.dt.int32, kind="ExternalInput")
    dst_t = nc.dram_tensor("edst", [P, k_total], f32, kind="ExternalInput")
    nrm_t = nc.dram_tensor("enrm", [P, k_total], f32, kind="ExternalInput")
    w_t = nc.dram_tensor("conv_w", [n_layers, P, P], f32, kind="ExternalInput")
    cb_t = nc.dram_tensor("conv_b", [n_layers, P], f32, kind="ExternalInput")
    gam_t = nc.dram_tensor("bn_gamma", [n_layers, P], f32, kind="ExternalInput")
    bet_t = nc.dram_tensor("bn_beta", [n_layers, P], f32, kind="ExternalInput")
    mu_t = nc.dram_tensor("bn_mean", [n_layers, P], f32, kind="ExternalInput")
    var_t = nc.dram_tensor("bn_var", [n_layers, P], f32, kind="ExternalInput")
    lw_t = nc.dram_tensor("lin_w", [P, n_cls], f32, kind="ExternalInput")
    lb_t = nc.dram_tensor("lin_b_rep", [P, n_cls], f32, kind="ExternalInput")
    out_t = nc.dram_tensor("out", [sh, n_cls], f32, kind="ExternalOutput")

    ag_in = [nc.dram_tensor(f"ag_in{l}", [sh, P], f32) for l in range(n_layers - 1)]
    hbuf = [nc.dram_tensor(f"hbuf{l}", [n_nodes, P], f32, addr_space="Shared")
            for l in range(n_layers - 1)]
    rgroups = [list(range(ncores))]
    AF = mybir.ActivationFunctionType
    OP = mybir.AluOpType

    with tile.TileContext(nc) as tc:
        with tc.tile_pool(name="const", bufs=1) as cpool, \
             tc.tile_pool(name="edges", bufs=1) as epool, \
             tc.tile_pool(name="msgs", bufs=8) as mpool, \
             tc.tile_pool(name="spool", bufs=4) as spool, \
             tc.tile_pool(name="work", bufs=3) as wpool, \
             tc.tile_pool(name="psum", bufs=2, space="PSUM") as pspool:

            # -------- resident edge data + constants
            idx_sb = epool.tile([P, k_total], mybir.dt.int32)
            dst_sb = epool.tile([P, k_total], f32)
            nrm_sb = epool.tile([P, k_total], f32)
            nc.sync.dma_start(out=idx_sb[:], in_=idx_t[:])
            nc.sync.dma_start(out=dst_sb[:], in_=dst_t[:])
            nc.sync.dma_start(out=nrm_sb[:], in_=nrm_t[:])

            iota_i = cpool.tile([P, P], mybir.dt.int32)
            nc.gpsimd.iota(iota_i[:], pattern=[[1, P]], base=0, channel_multiplier=0)
            iota_f = cpool.tile([P, P], f32)
            nc.vector.tensor_copy(iota_f[:], iota_i[:])
            ident = cpool.tile([P, P], f32)
            make_identity(nc, ident[:])

            w_sb = []
            for l in range(n_layers):
                wl = cpool.tile([P, P], f32, tag=f"w{l}")
                nc.sync.dma_start(out=wl[:], in_=w_t[l, :, :])
                w_sb.append(wl)
            lw_sb = cpool.tile([P, n_cls], f32)
            nc.sync.dma_start(out=lw_sb[:], in_=lw_t[:])
            lb_sb = cpool.tile([P, n_cls], f32)
            nc.sync.dma_start(out=lb_sb[:], in_=lb_t[:])

            # -------- BN constants per layer: scale s = gamma * rsqrt(var+eps)
            #          shift = s*(conv_b - mean) + beta      (feature-major [P,1])
            s_sb, sh_sb = [], []
            for l in range(n_layers):
                g_ = cpool.tile([P, 1], f32, tag=f"bng{l}")
                b_ = cpool.tile([P, 1], f32, tag=f"bnb{l}")
                m_ = cpool.tile([P, 1], f32, tag=f"bnm{l}")
                v_ = cpool.tile([P, 1], f32, tag=f"bnv{l}")
                cb_ = cpool.tile([P, 1], f32, tag=f"bnc{l}")
                nc.sync.dma_start(out=g_[:], in_=gam_t[l, :, None])
                nc.sync.dma_start(out=b_[:], in_=bet_t[l, :, None])
                nc.sync.dma_start(out=m_[:], in_=mu_t[l, :, None])
                nc.sync.dma_start(out=v_[:], in_=var_t[l, :, None])
                nc.sync.dma_start(out=cb_[:], in_=cb_t[l, :, None])
                ve = cpool.tile([P, 1], f32, tag=f"bnve{l}")
                nc.vector.tensor_scalar_add(ve[:], v_[:], BN_EPS)
                nc.scalar.sqrt(ve[:], ve[:])
                rv = cpool.tile([P, 1], f32, tag=f"bnrv{l}")
                nc.vector.reciprocal(rv[:], ve[:])
                s_ = cpool.tile([P, 1], f32, tag=f"bns{l}")
                nc.vector.tensor_tensor(out=s_[:], in0=g_[:], in1=rv[:], op=OP.mult)
                d_ = cpool.tile([P, 1], f32, tag=f"bnd{l}")
                nc.vector.tensor_tensor(out=d_[:], in0=cb_[:], in1=m_[:], op=OP.subtract)
                t_ = cpool.tile([P, 1], f32, tag=f"bnt{l}")
                nc.vector.tensor_tensor(out=t_[:], in0=d_[:], in1=s_[:], op=OP.mult)
                nc.vector.tensor_tensor(out=t_[:], in0=t_[:], in1=b_[:], op=OP.add)
                s_sb.append(s_)
                sh_sb.append(t_)

            hmax = epool.tile([P, t_tiles * P], f32)
            nc.vector.memset(hmax[:], 0.0)

            # -------- layers
            for l in range(n_layers):
                table = x_t if l == 0 else hbuf[l - 1]
                for grp in groups:
                    for t in grp:
                        kt = k_tiles[t]
                        psq = pspool.tile([P, P], f32, tag="q", space="PSUM")
                        for j in range(kt):
                            c = int(offs[t]) + j
                            # HW indirect DMA: one dynamic offset per output
                            # partition -> gather exactly 128 rows per call.
                            msgs = mpool.tile([P, P], f32, tag="msgs")
                            nc.gpsimd.indirect_dma_start(
                                out=msgs[:], out_offset=None,
                                in_=table[:],
                                in_offset=bass.IndirectOffsetOnAxis(
                                    ap=idx_sb[:, c:c + 1], axis=0),
                            )
                            s_tile = spool.tile([P, P], f32, tag="S")
                            nc.vector.tensor_scalar(
                                out=s_tile[:], in0=iota_f[:],
                                scalar1=dst_sb[:, c:c + 1],
                                scalar2=nrm_sb[:, c:c + 1],
                                op0=OP.is_equal, op1=OP.mult)
                            nc.tensor.matmul(
                                psq[:], lhsT=msgs[:],
                                rhs=s_tile[:], start=(j == 0), stop=(j == kt - 1))
                        q_sb = wpool.tile([P, P], f32, tag="qT")
                        nc.vector.tensor_copy(q_sb[:], psq[:])
                        ph = pspool.tile([P, P], f32, tag="h", space="PSUM")
                        nc.tensor.matmul(ph[:], lhsT=w_sb[l][:], rhs=q_sb[:],
                                         start=True, stop=True)
                        h_t = wpool.tile([P, P], f32, tag="hT")
                        nc.scalar.activation(h_t[:], ph[:], AF.Relu,
                                             bias=sh_sb[l][:, :1],
                                             scale=s_sb[l][:, :1])
                        nc.vector.tensor_tensor(
                            out=hmax[:, t * P:(t + 1) * P],
                            in0=hmax[:, t * P:(t + 1) * P], in1=h_t[:], op=OP.max)
                        if l < n_layers - 1:
                            pt = pspool.tile([P, P], f32, tag="t", space="PSUM")
                            nc.tensor.transpose(pt[:], h_t[:], ident[:])
                            hn = wpool.tile([P, P], f32, tag="hn")
                            nc.scalar.copy(hn[:], pt[:])
                            rows = min(P, sh - t * P)
                            nc.sync.dma_start(out=ag_in[l][t * P:t * P + rows, :],
                                              in_=hn[:rows, :])
                if l < n_layers - 1:
                    nc.gpsimd.collective_compute(
                        "AllGather", OP.bypass, replica_groups=rgroups,
                        ins=[ag_in[l][:]], outs=[hbuf[l][:]])

            # -------- head: logits + log_softmax
            for t in range(t_tiles):
                po = pspool.tile([P, n_cls], f32, tag="h", space="PSUM")
                nc.tensor.matmul(po[:], lhsT=hmax[:, t * P:(t + 1) * P],
                                 rhs=lw_sb[:], start=True, stop=True)
                z = wpool.tile([P, n_cls], f32, tag="z")
                nc.vector.tensor_tensor(out=z[:], in0=po[:], in1=lb_sb[:], op=OP.add)
                nm = wpool.tile([P, 1], f32, tag="nm")
                nc.vector.reduce_max(nm[:], z[:], axis=mybir.AxisListType.X,
                                     negate=True)
                ez = wpool.tile([P, n_cls], f32, tag="ez")
                nc.scalar.activation(ez[:], z[:], AF.Exp, bias=nm[:, :1], scale=1.0)
                ss = wpool.tile([P, 1], f32, tag="ss")
                nc.vector.reduce_sum(ss[:], ez[:], axis=mybir.AxisListType.X)
                ls = wpool.tile([P, 1], f32, tag="ls")
                nc.scalar.activation(ls[:], ss[:], AF.Ln)
                oz = wpool.tile([P, n_cls], f32, tag="oz")
                nc.vector.tensor_scalar(out=oz[:], in0=z[:],
                                        scalar1=nm[:, :1], scalar2=ls[:, :1],
                                        op0=OP.add, op1=OP.subtract)
                rows = min(P, sh - t * P)
                nc.sync.dma_start(out=out_t[t * P:t * P + rows, :],
                                  in_=oz[:rows, :])

    nc.compile()
    return nc


# ---------------------------------------------------------------- runner
def run(x, edge_index, conv_w, conv_b, bn_gamma, bn_beta, bn_mean, bn_var,
        lin_w, lin_b, *, trace=False):
    n_nodes, d = x.shape
    n_layers = conv_w.shape[0]
    n_cls = lin_w.shape[1]
    assert d == P and n_nodes % NCORES == 0

    per_core, k_tiles = preprocess_edges(edge_index, n_nodes)
    nc = build_program(n_nodes, n_layers, n_cls, k_tiles)

    shared = {
        "x": np.ascontiguousarray(np.asarray(x, dtype=np.float32)),
        "conv_w": np.ascontiguousarray(np.asarray(conv_w, dtype=np.float32)),
        "conv_b": np.ascontiguousarray(np.asarray(conv_b, dtype=np.float32)),
        "bn_gamma": np.ascontiguousarray(np.asarray(bn_gamma, dtype=np.float32)),
        "bn_beta": np.ascontiguousarray(np.asarray(bn_beta, dtype=np.float32)),
        "bn_mean": np.ascontiguousarray(np.asarray(bn_mean, dtype=np.float32)),
        "bn_var": np.ascontiguousarray(np.asarray(bn_var, dtype=np.float32)),
        "lin_w": np.ascontiguousarray(np.asarray(lin_w, dtype=np.float32)),
        "lin_b_rep": np.ascontiguousarray(
            np.broadcast_to(np.asarray(lin_b, dtype=np.float32), (P, n_cls))),
    }
    in_maps = [dict(shared, **per_core[c]) for c in range(NCORES)]
    res = run_bass_kernel_spmd(nc, in_maps, list(range(NCORES)), trace=trace)
    out = np.concatenate([np.asarray(res.results[c]["out"])
                          for c in range(NCORES)], axis=0)
    return out, res


def kernel(x, edge_index, conv_w, conv_b, bn_gamma, bn_beta, bn_mean, bn_var,
           lin_w, lin_b):
    out, _ = run(x, edge_index, conv_w, conv_b, bn_gamma, bn_beta,
                 bn_mean, bn_var, lin_w, lin_b,
                 trace=bool(int(os.environ.get("JKNET_TRACE", "0"))))
    return out

